# revision 6
# baseline (speedup 1.0000x reference)
"""MoE layer (router + hard gumbel gate + per-expert 2-layer MLP) on 8 Trainium2 cores.

Strategy
--------
- Routing (tiny FLOPs, but bitwise-sensitive through the hard gumbel-argmax
  gate): computed with in-process jax on the default backend, mirroring the
  reference line-for-line so argmax decisions and the aux losses match the
  oracle exactly.  The gumbel noise comes from jax.random.key(42) and is
  input-independent.
- Expert compute (99% of FLOPs): expert-parallel across the 8 NeuronCores.
  Core e gets the tokens routed to expert e (host gather, padded to a fixed
  capacity C), plus We1[e]/We2[e], and runs a Bass/Tile kernel computing
  relu(x @ We1 + be1) @ We2 + be2 with float32r matmuls (full PE rate,
  fp32 storage).  Outputs are scattered back to token order on the host.

The hard gate makes gates == one_hot(argmax) exactly (y_soft - y_soft == 0
elementwise in fp32), so only the selected expert's output is needed per
token.
"""

import sys

import numpy as np

try:
    import concourse  # noqa: F401
except ImportError:
    sys.path.insert(0, "/opt/trn_rl_repo")

N, D, H, E = 8192, 1024, 2048, 8
C_CAP = 1408  # per-expert token capacity per kernel round (11 tiles of 128)


# ---------------------------------------------------------------- routing ----
def _routing(x, Wr1, br1, Wr2, br2, expert_emb):
    """Mirror the reference router exactly (same jax ops, default backend)."""
    import jax
    import jax.numpy as jnp

    x = jnp.asarray(x)
    inp_emb = jax.nn.relu(x @ jnp.asarray(Wr1) + jnp.asarray(br1)) @ jnp.asarray(
        Wr2
    ) + jnp.asarray(br2)
    router_logits = inp_emb @ jnp.asarray(expert_emb).T

    gkey = jax.random.key(42)
    u = jax.random.uniform(gkey, router_logits.shape, minval=1e-6, maxval=1.0)
    gumbel = -jnp.log(-jnp.log(u))
    y_soft = jax.nn.softmax(router_logits + gumbel, axis=-1)
    assign = jnp.argmax(y_soft, axis=-1)
    y_hard = jax.nn.one_hot(assign, E, dtype=y_soft.dtype)
    gates = y_hard + y_soft - jax.lax.stop_gradient(y_soft)

    router_probs = jax.nn.softmax(router_logits, axis=-1)
    fraction_of_tokens = gates.mean(axis=0)
    mean_router_prob = router_probs.mean(axis=0)
    load_balancing_loss = E * jnp.sum(fraction_of_tokens * mean_router_prob)
    entropy = -jnp.mean(
        jnp.sum(router_probs * jnp.log(router_probs + 1e-9), axis=-1)
    )
    return (
        np.asarray(assign),
        np.asarray(gates, dtype=np.float32),
        np.asarray(load_balancing_loss),
        np.asarray(entropy),
    )


# ----------------------------------------------------------- bass program ----
_CACHE = {}


def _build_bass(C, repeat=1):
    from contextlib import ExitStack, nullcontext

    import concourse.tile as tile
    from concourse import bacc, mybir

    f32 = mybir.dt.float32
    f32r = mybir.dt.float32r
    Relu = mybir.ActivationFunctionType.Relu

    nc = bacc.Bacc("TRN2", target_bir_lowering=False, debug=False)
    xeT = nc.dram_tensor("xeT", [D, C], f32r, kind="ExternalInput")
    w1 = nc.dram_tensor("w1", [D, H], f32r, kind="ExternalInput")
    b1 = nc.dram_tensor("b1", [H], f32, kind="ExternalInput")
    w2 = nc.dram_tensor("w2", [H, D], f32r, kind="ExternalInput")
    b2 = nc.dram_tensor("b2", [D], f32r, kind="ExternalInput")
    ye = nc.dram_tensor("ye", [C, D], f32, kind="ExternalOutput")

    ND = D // 128  # 8 k-tiles over D
    NH = H // 128  # 16 stripes over H
    NCT = C // 128  # output row tiles
    # layer-1 column chunks over C (each <=512, all >=256 for full-rate f32r)
    chunks = []
    c0 = 0
    while c0 < C:
        cw = min(512, C - c0)
        chunks.append((c0, cw))
        c0 += cw

    with tile.TileContext(nc) as tc, ExitStack() as ctx:
        const_pool = ctx.enter_context(tc.tile_pool(name="const", bufs=1))
        b1_sb = const_pool.tile([128, NH], f32)
        nc.sync.dma_start(b1_sb[:], b1[:].rearrange("(h p) -> p h", p=128))
        b2_sb = const_pool.tile([1, D], f32r)
        nc.sync.dma_start(b2_sb[:], b2[:].rearrange("(o n) -> o n", o=1))
        ones_f32 = const_pool.tile([1, 128], f32)
        nc.vector.memset(ones_f32[:], 1.0)
        ones_sb = const_pool.tile([1, 128], f32r)
        nc.scalar.copy(ones_sb[:], ones_f32[:])

        ht_pool = ctx.enter_context(tc.tile_pool(name="ht", bufs=1))
        hT = ht_pool.tile([128, NH, C], f32r)

        psum_pool = ctx.enter_context(
            tc.tile_pool(name="psum", bufs=8, space="PSUM")
        )

        loop = tc.For_i(0, repeat, 1) if repeat > 1 else nullcontext()
        ctx.enter_context(loop)

        # ---- layer 1: hT[h,:] = relu(w1.T @ xeT + b1) ----
        with (
            tc.tile_pool(name="we1", bufs=1) as we1_pool,
            tc.tile_pool(name="xe", bufs=2) as xe_pool,
        ):
            we1_sb = we1_pool.tile([128, ND, H], f32r)
            for d in range(ND):
                nc.sync.dma_start(we1_sb[:, d, :], w1[d * 128 : (d + 1) * 128, :])
            for c0, cw in chunks:
                xt = xe_pool.tile([128, ND, 512], f32r, tag="xe")
                for d in range(ND):
                    nc.sync.dma_start(
                        xt[:, d, :cw], xeT[d * 128 : (d + 1) * 128, c0 : c0 + cw]
                    )
                for h in range(NH):
                    ps = psum_pool.tile([128, 512], f32)
                    for d in range(ND):
                        nc.tensor.matmul(
                            ps[:, :cw],
                            we1_sb[:, d, h * 128 : (h + 1) * 128],
                            xt[:, d, :cw],
                            start=(d == 0),
                            stop=(d == ND - 1),
                        )
                    nc.scalar.activation(
                        hT[:, h, c0 : c0 + cw],
                        ps[:, :cw],
                        Relu,
                        bias=b1_sb[:, h : h + 1],
                        scale=1.0,
                    )

        # ---- layer 2: ye = hT.T @ w2 + b2 ----
        with (
            tc.tile_pool(name="we2", bufs=1) as we2_pool,
            tc.tile_pool(name="out", bufs=3) as out_pool,
        ):
            we2_sb = we2_pool.tile([128, NH, D], f32r)
            for h in range(NH):
                nc.sync.dma_start(we2_sb[:, h, :], w2[h * 128 : (h + 1) * 128, :])
            for ct in range(NCT):
                ot = out_pool.tile([128, D], f32)
                for dh in range(2):
                    ps = psum_pool.tile([128, 512], f32)
                    nc.tensor.matmul(
                        ps[:],
                        ones_sb[0:1, :],
                        b2_sb[0:1, dh * 512 : (dh + 1) * 512],
                        start=True,
                        stop=False,
                    )
                    for h in range(NH):
                        nc.tensor.matmul(
                            ps[:],
                            hT[:, h, ct * 128 : (ct + 1) * 128],
                            we2_sb[:, h, dh * 512 : (dh + 1) * 512],
                            start=False,
                            stop=(h == NH - 1),
                        )
                    nc.vector.tensor_copy(ot[:, dh * 512 : (dh + 1) * 512], ps[:])
                nc.sync.dma_start(ye[ct * 128 : (ct + 1) * 128, :], ot[:])

    nc.compile()
    return nc


def _get_bass(C):
    if C not in _CACHE:
        _CACHE[C] = _build_bass(C)
    return _CACHE[C]


# ----------------------------------------------------------------- kernel ----
def kernel(x, Wr1, br1, Wr2, br2, expert_emb, We1, be1, We2, be2):
    from concourse import bass_utils

    x = np.ascontiguousarray(np.asarray(x, dtype=np.float32))
    We1 = np.ascontiguousarray(np.asarray(We1, dtype=np.float32))
    be1 = np.ascontiguousarray(np.asarray(be1, dtype=np.float32))
    We2 = np.ascontiguousarray(np.asarray(We2, dtype=np.float32))
    be2 = np.ascontiguousarray(np.asarray(be2, dtype=np.float32))

    assign, gates, lbl, ent = _routing(x, Wr1, br1, Wr2, br2, expert_emb)
    idx = [np.nonzero(assign == e)[0] for e in range(E)]

    nc = _get_bass(C_CAP)
    y = np.zeros((N, D), dtype=np.float32)
    rounds = max(1, max((len(i) + C_CAP - 1) // C_CAP for i in idx))
    for r in range(rounds):
        in_maps = []
        for e in range(E):
            ids = idx[e][r * C_CAP : (r + 1) * C_CAP]
            xeT = np.zeros((D, C_CAP), dtype=np.float32)
            if len(ids):
                xeT[:, : len(ids)] = x[ids].T
            in_maps.append(
                {
                    "xeT": xeT,
                    "w1": We1[e],
                    "b1": be1[e],
                    "w2": We2[e],
                    "b2": be2[e],
                }
            )
        res = bass_utils.run_bass_kernel_spmd(nc, in_maps, core_ids=list(range(E)))
        for e in range(E):
            ids = idx[e][r * C_CAP : (r + 1) * C_CAP]
            if len(ids):
                # scale by the gate value (== 1.0 up to fp32 rounding of the
                # straight-through estimator, reference computes (1+p)-p)
                y[ids] = res.results[e]["ye"][: len(ids)] * gates[
                    ids, e, None
                ]
    return y, lbl, ent


# revision 13
# speedup vs baseline: 2.2778x; 2.2778x over previous
"""MoE layer (router + hard gumbel gate + per-expert 2-layer MLP) on 8 Trainium2 cores.

Strategy
--------
- Routing (tiny FLOPs, but bitwise-sensitive through the hard gumbel-argmax
  gate): computed with in-process jax on the default backend, mirroring the
  reference line-for-line so argmax decisions and the aux losses match the
  oracle exactly.  The gumbel noise comes from jax.random.key(42) and is
  input-independent.
- Expert compute (99% of FLOPs): expert-parallel across the 8 NeuronCores.
  Core e gets the tokens routed to expert e (host gather, padded to a fixed
  capacity C), plus We1[e]/We2[e], and runs a Bass/Tile kernel computing
  relu(x @ We1 + be1) @ We2 + be2 with float32r matmuls (full PE rate,
  fp32 storage).  Outputs are scattered back to token order on the host.

The hard gate makes gates == one_hot(argmax) exactly (y_soft - y_soft == 0
elementwise in fp32), so only the selected expert's output is needed per
token.
"""

import sys

import numpy as np

try:
    import concourse  # noqa: F401
except ImportError:
    sys.path.insert(0, "/opt/trn_rl_repo")

N, D, H, E = 8192, 1024, 2048, 8
C_CAP = 1408  # per-expert token capacity per kernel round (11 tiles of 128)


# ---------------------------------------------------------------- routing ----
def _routing(x, Wr1, br1, Wr2, br2, expert_emb):
    """Mirror the reference router exactly (same jax ops, default backend)."""
    import jax
    import jax.numpy as jnp

    x = jnp.asarray(x)
    inp_emb = jax.nn.relu(x @ jnp.asarray(Wr1) + jnp.asarray(br1)) @ jnp.asarray(
        Wr2
    ) + jnp.asarray(br2)
    router_logits = inp_emb @ jnp.asarray(expert_emb).T

    gkey = jax.random.key(42)
    u = jax.random.uniform(gkey, router_logits.shape, minval=1e-6, maxval=1.0)
    gumbel = -jnp.log(-jnp.log(u))
    y_soft = jax.nn.softmax(router_logits + gumbel, axis=-1)
    assign = jnp.argmax(y_soft, axis=-1)
    y_hard = jax.nn.one_hot(assign, E, dtype=y_soft.dtype)
    gates = y_hard + y_soft - jax.lax.stop_gradient(y_soft)

    router_probs = jax.nn.softmax(router_logits, axis=-1)
    fraction_of_tokens = gates.mean(axis=0)
    mean_router_prob = router_probs.mean(axis=0)
    load_balancing_loss = E * jnp.sum(fraction_of_tokens * mean_router_prob)
    entropy = -jnp.mean(
        jnp.sum(router_probs * jnp.log(router_probs + 1e-9), axis=-1)
    )
    return (
        np.asarray(assign),
        np.asarray(gates, dtype=np.float32),
        np.asarray(load_balancing_loss),
        np.asarray(entropy),
    )


# ----------------------------------------------------------- bass program ----
_CACHE = {}


MM_DT = "float16"  # matmul storage dtype: "float32r" or "float16"


def _build_bass(C, repeat=1, mm_dt=None):
    from contextlib import ExitStack, nullcontext

    import concourse.tile as tile
    from concourse import bacc, mybir

    f32 = mybir.dt.float32
    fmm = getattr(mybir.dt, mm_dt or MM_DT)
    Relu = mybir.ActivationFunctionType.Relu

    nc = bacc.Bacc("TRN2", target_bir_lowering=False, debug=False)
    xeT = nc.dram_tensor("xeT", [D, C], fmm, kind="ExternalInput")
    w1 = nc.dram_tensor("w1", [D, H], fmm, kind="ExternalInput")
    b1 = nc.dram_tensor("b1", [H], f32, kind="ExternalInput")
    w2 = nc.dram_tensor("w2", [H, D], fmm, kind="ExternalInput")
    b2 = nc.dram_tensor("b2", [D], f32, kind="ExternalInput")
    ye = nc.dram_tensor("ye", [C, D], f32, kind="ExternalOutput")

    ND = D // 128  # 8 k-tiles over D
    NH = H // 128  # 16 stripes over H
    NCT = C // 128  # output row tiles
    # layer-1 column chunks over C (each <=512, all >=256 for full-rate f32r)
    chunks = []
    c0 = 0
    while c0 < C:
        cw = min(512, C - c0)
        chunks.append((c0, cw))
        c0 += cw

    with tile.TileContext(nc) as tc, ExitStack() as ctx:
        const_pool = ctx.enter_context(tc.tile_pool(name="const", bufs=1))
        b1_sb = const_pool.tile([128, NH], f32)
        nc.sync.dma_start(b1_sb[:], b1[:].rearrange("(h p) -> p h", p=128))
        # b2 broadcast across all 128 partitions (for the layer-2 bias add)
        b2_bc = const_pool.tile([128, D], f32)
        nc.sync.dma_start(
            b2_bc[:], b2[:].rearrange("(o n) -> o n", o=1).broadcast_to((128, D))
        )

        # both weight sets stay resident (fp16), so layer-2 of chunk k can
        # follow layer-1 of chunk k with no mid-kernel weight-load bubble
        wpool = ctx.enter_context(tc.tile_pool(name="w", bufs=1))
        we1_sb = wpool.tile([128, ND, H], fmm)
        we2_sb = wpool.tile([128, NH, D], fmm)

        ht_pool = ctx.enter_context(tc.tile_pool(name="ht", bufs=2))
        xe_pool = ctx.enter_context(tc.tile_pool(name="xe", bufs=2))
        out_pool = ctx.enter_context(tc.tile_pool(name="out", bufs=3))
        psum_pool = ctx.enter_context(
            tc.tile_pool(name="psum", bufs=8, space="PSUM")
        )

        loop = tc.For_i(0, repeat, 1) if repeat > 1 else nullcontext()
        with loop:
            for d in range(ND):
                nc.sync.dma_start(we1_sb[:, d, :], w1[d * 128 : (d + 1) * 128, :])
            for h in range(NH):
                nc.sync.dma_start(we2_sb[:, h, :], w2[h * 128 : (h + 1) * 128, :])

            for c0, cw in chunks:
                xt = xe_pool.tile([128, ND, 512], fmm, tag="xe")
                for d in range(ND):
                    nc.sync.dma_start(
                        xt[:, d, :cw], xeT[d * 128 : (d + 1) * 128, c0 : c0 + cw]
                    )
                htc = ht_pool.tile([128, NH, 512], fmm, tag="ht")
                # layer 1: htc[h,:] = relu(w1.T @ x + b1), H on partitions
                for h in range(NH):
                    ps = psum_pool.tile([128, 512], f32)
                    for d in range(ND):
                        nc.tensor.matmul(
                            ps[:, :cw],
                            we1_sb[:, d, h * 128 : (h + 1) * 128],
                            xt[:, d, :cw],
                            start=(d == 0),
                            stop=(d == ND - 1),
                        )
                    nc.scalar.activation(
                        htc[:, h, :cw],
                        ps[:, :cw],
                        Relu,
                        bias=b1_sb[:, h : h + 1],
                        scale=1.0,
                    )
                # layer 2: ye rows = htc.T @ w2 + b2, tokens on partitions
                for ct in range(cw // 128):
                    ot = out_pool.tile([128, D], f32)
                    for dh in range(2):
                        ps = psum_pool.tile([128, 512], f32)
                        for h in range(NH):
                            nc.tensor.matmul(
                                ps[:],
                                htc[:, h, ct * 128 : (ct + 1) * 128],
                                we2_sb[:, h, dh * 512 : (dh + 1) * 512],
                                start=(h == 0),
                                stop=(h == NH - 1),
                            )
                        nc.vector.tensor_add(
                            ot[:, dh * 512 : (dh + 1) * 512],
                            ps[:],
                            b2_bc[:, dh * 512 : (dh + 1) * 512],
                        )
                    nc.sync.dma_start(
                        ye[c0 + ct * 128 : c0 + (ct + 1) * 128, :], ot[:]
                    )

    nc.compile()
    return nc


def _get_bass(C):
    if C not in _CACHE:
        _CACHE[C] = _build_bass(C)
    return _CACHE[C]


# ----------------------------------------------------------------- kernel ----
def kernel(x, Wr1, br1, Wr2, br2, expert_emb, We1, be1, We2, be2):
    from concourse import bass_utils

    x = np.ascontiguousarray(np.asarray(x, dtype=np.float32))
    We1 = np.ascontiguousarray(np.asarray(We1, dtype=np.float32))
    be1 = np.ascontiguousarray(np.asarray(be1, dtype=np.float32))
    We2 = np.ascontiguousarray(np.asarray(We2, dtype=np.float32))
    be2 = np.ascontiguousarray(np.asarray(be2, dtype=np.float32))

    assign, gates, lbl, ent = _routing(x, Wr1, br1, Wr2, br2, expert_emb)
    idx = [np.nonzero(assign == e)[0] for e in range(E)]

    nc = _get_bass(C_CAP)
    mm_np = np.float16 if MM_DT == "float16" else np.float32
    x_mm = x.astype(mm_np)
    We1_mm = We1.astype(mm_np)
    We2_mm = We2.astype(mm_np)
    y = np.zeros((N, D), dtype=np.float32)
    rounds = max(1, max((len(i) + C_CAP - 1) // C_CAP for i in idx))
    for r in range(rounds):
        in_maps = []
        for e in range(E):
            ids = idx[e][r * C_CAP : (r + 1) * C_CAP]
            xeT = np.zeros((D, C_CAP), dtype=mm_np)
            if len(ids):
                xeT[:, : len(ids)] = x_mm[ids].T
            in_maps.append(
                {
                    "xeT": xeT,
                    "w1": We1_mm[e],
                    "b1": be1[e],
                    "w2": We2_mm[e],
                    "b2": be2[e],
                }
            )
        res = bass_utils.run_bass_kernel_spmd(nc, in_maps, core_ids=list(range(E)))
        for e in range(E):
            ids = idx[e][r * C_CAP : (r + 1) * C_CAP]
            if len(ids):
                # scale by the gate value (== 1.0 up to fp32 rounding of the
                # straight-through estimator, reference computes (1+p)-p)
                y[ids] = res.results[e]["ye"][: len(ids)] * gates[
                    ids, e, None
                ]
    return y, lbl, ent


# revision 17
# speedup vs baseline: 2.5540x; 1.1213x over previous
"""MoE layer (router + hard gumbel gate + per-expert 2-layer MLP) on 8 Trainium2 cores.

Strategy
--------
- Routing (tiny FLOPs, but bitwise-sensitive through the hard gumbel-argmax
  gate): computed with in-process jax on the default backend, mirroring the
  reference line-for-line so argmax decisions and the aux losses match the
  oracle exactly.  The gumbel noise comes from jax.random.key(42) and is
  input-independent.
- Expert compute (99% of FLOPs): expert-parallel across the 8 NeuronCores.
  Core e gets the tokens routed to expert e (host gather, padded to a fixed
  capacity C), plus We1[e]/We2[e], and runs a Bass/Tile kernel computing
  relu(x @ We1 + be1) @ We2 + be2 with float16 matmuls (full PE rate;
  fp32 PSUM accumulation, fp32 bias/output).  Outputs are scattered back
  to token order on the host.

The hard gate makes gates == one_hot(argmax) exactly (y_soft - y_soft == 0
elementwise in fp32), so only the selected expert's output is needed per
token.
"""

import sys

import numpy as np

try:
    import concourse  # noqa: F401
except ImportError:
    sys.path.insert(0, "/opt/trn_rl_repo")

N, D, H, E = 8192, 1024, 2048, 8


def _capacity(counts):
    """Per-expert token capacity: max count rounded up to a 128 row tile."""
    m = max(1, max(counts))
    return -(-m // 128) * 128


# ---------------------------------------------------------------- routing ----
def _routing(x, Wr1, br1, Wr2, br2, expert_emb):
    """Mirror the reference router exactly (same jax ops, default backend)."""
    import jax
    import jax.numpy as jnp

    x = jnp.asarray(x)
    inp_emb = jax.nn.relu(x @ jnp.asarray(Wr1) + jnp.asarray(br1)) @ jnp.asarray(
        Wr2
    ) + jnp.asarray(br2)
    router_logits = inp_emb @ jnp.asarray(expert_emb).T

    gkey = jax.random.key(42)
    u = jax.random.uniform(gkey, router_logits.shape, minval=1e-6, maxval=1.0)
    gumbel = -jnp.log(-jnp.log(u))
    y_soft = jax.nn.softmax(router_logits + gumbel, axis=-1)
    assign = jnp.argmax(y_soft, axis=-1)
    y_hard = jax.nn.one_hot(assign, E, dtype=y_soft.dtype)
    gates = y_hard + y_soft - jax.lax.stop_gradient(y_soft)

    router_probs = jax.nn.softmax(router_logits, axis=-1)
    fraction_of_tokens = gates.mean(axis=0)
    mean_router_prob = router_probs.mean(axis=0)
    load_balancing_loss = E * jnp.sum(fraction_of_tokens * mean_router_prob)
    entropy = -jnp.mean(
        jnp.sum(router_probs * jnp.log(router_probs + 1e-9), axis=-1)
    )
    return (
        np.asarray(assign),
        np.asarray(gates, dtype=np.float32),
        np.asarray(load_balancing_loss),
        np.asarray(entropy),
    )


# ----------------------------------------------------------- bass program ----
_CACHE = {}


MM_DT = "float16"  # matmul storage dtype: "float32r" or "float16"


def _build_bass(C, repeat=1, mm_dt=None):
    from contextlib import ExitStack, nullcontext

    import concourse.tile as tile
    from concourse import bacc, mybir

    f32 = mybir.dt.float32
    fmm = getattr(mybir.dt, mm_dt or MM_DT)
    Relu = mybir.ActivationFunctionType.Relu

    nc = bacc.Bacc("TRN2", target_bir_lowering=False, debug=False)
    xeT = nc.dram_tensor("xeT", [D, C], fmm, kind="ExternalInput")
    w1 = nc.dram_tensor("w1", [D, H], fmm, kind="ExternalInput")
    b1 = nc.dram_tensor("b1", [H], f32, kind="ExternalInput")
    w2 = nc.dram_tensor("w2", [H, D], fmm, kind="ExternalInput")
    b2 = nc.dram_tensor("b2", [D], f32, kind="ExternalInput")
    ye = nc.dram_tensor("ye", [C, D], f32, kind="ExternalOutput")

    ND = D // 128  # 8 k-tiles over D
    NH = H // 128  # 16 stripes over H
    NCT = C // 128  # output row tiles
    # layer-1 column chunks over C (<=512 each: one fp32 PSUM bank)
    chunks = []
    c0 = 0
    while c0 < C:
        cw = min(512, C - c0)
        chunks.append((c0, cw))
        c0 += cw

    with tile.TileContext(nc) as tc, ExitStack() as ctx:
        const_pool = ctx.enter_context(tc.tile_pool(name="const", bufs=1))
        b1_sb = const_pool.tile([128, NH], f32)
        nc.sync.dma_start(b1_sb[:], b1[:].rearrange("(h p) -> p h", p=128))
        # b2 broadcast across all 128 partitions (for the layer-2 bias add)
        b2_bc = const_pool.tile([128, D], f32)
        nc.sync.dma_start(
            b2_bc[:], b2[:].rearrange("(o n) -> o n", o=1).broadcast_to((128, D))
        )

        # both weight sets stay resident (fp16), so layer-2 of chunk k can
        # follow layer-1 of chunk k with no mid-kernel weight-load bubble
        wpool = ctx.enter_context(tc.tile_pool(name="w", bufs=1))
        we1_sb = wpool.tile([128, ND, H], fmm)
        we2_sb = wpool.tile([128, NH, D], fmm)

        ht_pool = ctx.enter_context(tc.tile_pool(name="ht", bufs=2))
        xe_pool = ctx.enter_context(tc.tile_pool(name="xe", bufs=2))
        out_pool = ctx.enter_context(tc.tile_pool(name="out", bufs=3))
        psum_pool = ctx.enter_context(
            tc.tile_pool(name="psum", bufs=8, space="PSUM")
        )

        loop = tc.For_i(0, repeat, 1) if repeat > 1 else nullcontext()
        with loop:
            for d in range(ND):
                nc.sync.dma_start(we1_sb[:, d, :], w1[d * 128 : (d + 1) * 128, :])
            for h in range(NH):
                nc.sync.dma_start(we2_sb[:, h, :], w2[h * 128 : (h + 1) * 128, :])

            for c0, cw in chunks:
                xt = xe_pool.tile([128, ND, 512], fmm, tag="xe")
                for d in range(ND):
                    nc.sync.dma_start(
                        xt[:, d, :cw], xeT[d * 128 : (d + 1) * 128, c0 : c0 + cw]
                    )
                htc = ht_pool.tile([128, NH, 512], fmm, tag="ht")
                # layer 1: htc[h,:] = relu(w1.T @ x + b1), H on partitions
                for h in range(NH):
                    ps = psum_pool.tile([128, 512], f32)
                    for d in range(ND):
                        nc.tensor.matmul(
                            ps[:, :cw],
                            we1_sb[:, d, h * 128 : (h + 1) * 128],
                            xt[:, d, :cw],
                            start=(d == 0),
                            stop=(d == ND - 1),
                        )
                    nc.scalar.activation(
                        htc[:, h, :cw],
                        ps[:, :cw],
                        Relu,
                        bias=b1_sb[:, h : h + 1],
                        scale=1.0,
                    )
                # layer 2: ye rows = htc.T @ w2 + b2, tokens on partitions
                for ct in range(cw // 128):
                    ot = out_pool.tile([128, D], f32)
                    for dh in range(2):
                        ps = psum_pool.tile([128, 512], f32)
                        for h in range(NH):
                            nc.tensor.matmul(
                                ps[:],
                                htc[:, h, ct * 128 : (ct + 1) * 128],
                                we2_sb[:, h, dh * 512 : (dh + 1) * 512],
                                start=(h == 0),
                                stop=(h == NH - 1),
                            )
                        nc.vector.tensor_add(
                            ot[:, dh * 512 : (dh + 1) * 512],
                            ps[:],
                            b2_bc[:, dh * 512 : (dh + 1) * 512],
                        )
                    nc.sync.dma_start(
                        ye[c0 + ct * 128 : c0 + (ct + 1) * 128, :], ot[:]
                    )

    nc.compile()
    return nc


def _get_bass(C):
    if C not in _CACHE:
        _CACHE[C] = _build_bass(C)
    return _CACHE[C]


# ----------------------------------------------------------------- kernel ----
def kernel(x, Wr1, br1, Wr2, br2, expert_emb, We1, be1, We2, be2):
    from concourse import bass_utils

    x = np.ascontiguousarray(np.asarray(x, dtype=np.float32))
    We1 = np.ascontiguousarray(np.asarray(We1, dtype=np.float32))
    be1 = np.ascontiguousarray(np.asarray(be1, dtype=np.float32))
    We2 = np.ascontiguousarray(np.asarray(We2, dtype=np.float32))
    be2 = np.ascontiguousarray(np.asarray(be2, dtype=np.float32))

    assign, gates, lbl, ent = _routing(x, Wr1, br1, Wr2, br2, expert_emb)
    idx = [np.nonzero(assign == e)[0] for e in range(E)]

    C = _capacity([len(i) for i in idx])
    nc = _get_bass(C)
    mm_np = np.float16 if MM_DT == "float16" else np.float32
    x_mm = x.astype(mm_np)
    We1_mm = We1.astype(mm_np)
    We2_mm = We2.astype(mm_np)
    y = np.zeros((N, D), dtype=np.float32)
    rounds = max(1, max((len(i) + C - 1) // C for i in idx))
    for r in range(rounds):
        in_maps = []
        for e in range(E):
            ids = idx[e][r * C : (r + 1) * C]
            xeT = np.zeros((D, C), dtype=mm_np)
            if len(ids):
                xeT[:, : len(ids)] = x_mm[ids].T
            in_maps.append(
                {
                    "xeT": xeT,
                    "w1": We1_mm[e],
                    "b1": be1[e],
                    "w2": We2_mm[e],
                    "b2": be2[e],
                }
            )
        res = bass_utils.run_bass_kernel_spmd(nc, in_maps, core_ids=list(range(E)))
        for e in range(E):
            ids = idx[e][r * C : (r + 1) * C]
            if len(ids):
                # scale by the gate value (== 1.0 up to fp32 rounding of the
                # straight-through estimator, reference computes (1+p)-p)
                y[ids] = res.results[e]["ye"][: len(ids)] * gates[
                    ids, e, None
                ]
    return y, lbl, ent
